# revision 1
# baseline (speedup 1.0000x reference)
"""Trainium2 Bass kernel for nn_CrossAttention (B=2, Lq=Lkv=2048, E=1024, H=16, D=64).

Sharding: tensor-parallel over heads. Each of the 8 cores owns 2 heads
(a 128-wide slice of the QKV projection output and the matching 128
columns of Wo). Per core:

  phase P: Q^T/K^T/V^T projections (contraction over E in 8 chunks of
           128, fp32r matmuls at full PE rate), biases fused into the
           PSUM->SBUF copy on ScalarE.
  phase T: V^T -> V via PE transposes; V stored as [kpart, chunk,
           [1|h0|1|h1]] so a ones column rides along as stationary
           column 0, making each context matmul also produce the
           softmax denominator in PSUM row 0.
  phase A: flash-style attention per (batch, 512-wide q tile):
           scores^T = K^T.T @ Q^T with 64-row PE tiling (head0 on
           partitions 0-63, head1 on 64-127, concurrent); exp+mask via
           one ScalarE activation (scale=1/8, per-partition additive
           mask bias) straight from PSUM; context accumulated over the
           16 k chunks into 4 PSUM banks (2 row-tiles x 2 heads);
           denominator division via reciprocal + K=1 broadcast matmul;
           SBUF->SBUF DMA assembles ctx into [128 j, t] layout.
  phase O: out^T partial = Wo_c^T.T @ ctx, written to DRAM; host sums
           the 8 partials (the row-parallel Wo all-reduce).
"""

import sys

if "/opt/trn_rl_repo" not in sys.path:
    sys.path.insert(0, "/opt/trn_rl_repo")

import numpy as np

import concourse.tile as tile
from concourse import bacc, mybir
from concourse.bass_utils import run_bass_kernel_spmd
from concourse.masks import make_identity

F32 = mybir.dt.float32
F32R = mybir.dt.float32r
AF = mybir.ActivationFunctionType

N_CORES = 8
B, LQ, LKV, E, H, D = 2, 2048, 2048, 1024, 16, 64
HC = H // N_CORES  # heads per core = 2
JC = HC * D  # feature slice per core = 128
T = B * LQ  # 4096 tokens
NEC = E // 128  # 8 e-chunks
NTT = T // 512  # 8 token tiles of 512
NQT = LQ // 512  # 4 q tiles per batch
NKT = LKV // 128  # 16 k chunks per batch
NOC = E // 128  # 8 output chunks

_NC_CACHE = {}


def build(reps=None, phases="PTAO"):
    key = (reps or 0, phases)
    if key in _NC_CACHE:
        return _NC_CACHE[key]
    nc = bacc.Bacc("TRN2", target_bir_lowering=False, debug=False, num_devices=N_CORES)

    xqT = nc.dram_tensor("xqT", [E, T], F32R, kind="ExternalInput").ap()
    xkT = nc.dram_tensor("xkT", [E, T], F32R, kind="ExternalInput").ap()
    wqT = nc.dram_tensor("wqT", [E, JC], F32R, kind="ExternalInput").ap()
    wkT = nc.dram_tensor("wkT", [E, JC], F32R, kind="ExternalInput").ap()
    wvT = nc.dram_tensor("wvT", [E, JC], F32R, kind="ExternalInput").ap()
    woT = nc.dram_tensor("woT", [JC, E], F32R, kind="ExternalInput").ap()
    bqd = nc.dram_tensor("bq", [JC, 1], F32, kind="ExternalInput").ap()
    bkd = nc.dram_tensor("bk", [JC, 1], F32, kind="ExternalInput").ap()
    bvd = nc.dram_tensor("bv", [JC, 1], F32, kind="ExternalInput").ap()
    bod = nc.dram_tensor("bo", [NOC, 128], F32, kind="ExternalInput").ap()
    mbd = nc.dram_tensor("mb", [B, NKT, 128], F32, kind="ExternalInput").ap()
    outT = nc.dram_tensor("outT", [E, T], F32, kind="ExternalOutput").ap()

    from contextlib import nullcontext

    with tile.TileContext(nc) as tc, nc.allow_low_precision(reason="fp32r matmuls"):
        with tc.For_i(0, reps, 1) if reps else nullcontext():
         with (
             tc.tile_pool(name="const", bufs=1) as const,
             tc.tile_pool(name="big", bufs=1) as big,
         ):
             # ---- persistent SBUF state ----
             wq_sb = const.tile([128, NEC, JC], F32R, tag="wq")
             nc.sync.dma_start(out=wq_sb, in_=wqT.rearrange("(ec p) j -> p ec j", p=128))
             wk_sb = const.tile([128, NEC, JC], F32R, tag="wk")
             nc.sync.dma_start(out=wk_sb, in_=wkT.rearrange("(ec p) j -> p ec j", p=128))
             wv_sb = const.tile([128, NEC, JC], F32R, tag="wv")
             nc.sync.dma_start(out=wv_sb, in_=wvT.rearrange("(ec p) j -> p ec j", p=128))
             wo_sb = const.tile([128, NOC, 128], F32R, tag="wo")
             nc.sync.dma_start(out=wo_sb, in_=woT.rearrange("p (oc o) -> p oc o", oc=NOC))
             bq_sb = const.tile([128, 1], F32, tag="bq")
             nc.sync.dma_start(out=bq_sb, in_=bqd)
             bk_sb = const.tile([128, 1], F32, tag="bk")
             nc.sync.dma_start(out=bk_sb, in_=bkd)
             bv_sb = const.tile([128, 1], F32, tag="bv")
             nc.sync.dma_start(out=bv_sb, in_=bvd)
             bo_sb = const.tile([128, NOC], F32, tag="bo")
             nc.sync.dma_start(out=bo_sb, in_=bod.rearrange("oc o -> o oc"))
             mb_sb = const.tile([128, B, NKT], F32, tag="mb")
             nc.sync.dma_start(out=mb_sb, in_=mbd.rearrange("b kc p -> p b kc"))
             ident = const.tile([128, 128], F32, tag="ident")
             make_identity(nc, ident)
             ones_f = const.tile([1, 65], F32, tag="onesf")
             nc.vector.memset(ones_f, 1.0)
             onesc = const.tile([1, 65], F32R, tag="onesc")
             nc.vector.tensor_copy(onesc, ones_f)
             onecol = const.tile([128, 1], F32, tag="onecol")
             nc.vector.memset(onecol, 1.0)

             qt_sb = big.tile([128, T], F32R, tag="qt")
             kt_sb = big.tile([128, T], F32R, tag="kt")
             vt_sb = big.tile([128, T], F32, tag="vt")
             v_sb = big.tile([128, B * NKT, 130], F32R, tag="v")
             ctx_sb = big.tile([128, NTT, 512], F32R, tag="ctx")

             # ---- phase P: projections ----
             if "P" in phases:
              with (
                 tc.tile_pool(name="xin", bufs=2) as xin,
                 tc.tile_pool(name="pp", bufs=3, space="PSUM") as pp,
             ):
                 for xsrc, wsb, bias, dst, isv in (
                     (xqT, wq_sb, bq_sb, qt_sb, False),
                     (xkT, wk_sb, bk_sb, kt_sb, False),
                     (xkT, wv_sb, bv_sb, vt_sb, True),
                 ):
                     for tt in range(NTT):
                         xt = xin.tile([128, NEC, 512], F32R, tag="xin")
                         nc.sync.dma_start(
                             out=xt,
                             in_=xsrc[:, tt * 512 : (tt + 1) * 512].rearrange(
                                 "(ec p) t -> p ec t", p=128
                             ),
                         )
                         pt = pp.tile([128, 512], F32, tag="pp")
                         for ec in range(NEC):
                             nc.tensor.matmul(
                                 pt,
                                 wsb[:, ec, :],
                                 xt[:, ec, :],
                                 start=(ec == 0),
                                 stop=(ec == NEC - 1),
                             )
                         nc.scalar.activation(
                             out=dst[:, tt * 512 : (tt + 1) * 512],
                             in_=pt,
                             func=AF.Identity,
                             bias=bias,
                             scale=1.0,
                         )

             # ---- phase T: V transpose into [kpart, chunk, [1|h0|1|h1]] ----
             if "T" in phases:
              with tc.tile_pool(name="tp", bufs=3, space="PSUM") as tp:
                 for gc in range(B * NKT):
                     tpt = tp.tile([128, 128], F32, tag="tp")
                     nc.tensor.transpose(
                         tpt, vt_sb[:, gc * 128 : (gc + 1) * 128], ident
                     )
                     nc.vector.tensor_copy(v_sb[:, gc, 1:65], tpt[:, 0:64])
                     nc.vector.tensor_copy(v_sb[:, gc, 66:130], tpt[:, 64:128])
                     nc.vector.tensor_copy(v_sb[:, gc, 0:1], onecol)
                     nc.vector.tensor_copy(v_sb[:, gc, 65:66], onecol)

             # ---- phase A: attention ----
             if "A" in phases:
              with (
                 tc.tile_pool(name="attps", bufs=2, space="PSUM") as attps,
                 tc.tile_pool(name="cxps", bufs=1, space="PSUM") as cxps,
                 tc.tile_pool(name="expm", bufs=3) as expm,
                 tc.tile_pool(name="dv", bufs=2) as dv,
             ):
                 for b in range(B):
                     for qt in range(NQT):
                         q0 = b * LQ + qt * 512
                         cxs = [
                             cxps.tile([65, 512], F32, tag=f"cx{i}", name=f"cx{i}_{b}_{qt}")
                             for i in range(4)
                         ]
                         for kt in range(NKT):
                             k0 = b * LKV + kt * 128
                             sct = attps.tile([128, 2, 512], F32, tag="sc")
                             nc.tensor.matmul(
                                 sct[:, 0, :],
                                 kt_sb[0:64, k0 : k0 + 128],
                                 qt_sb[0:64, q0 : q0 + 512],
                                 start=True,
                                 stop=True,
                             )
                             nc.tensor.matmul(
                                 sct[:, 1, :],
                                 kt_sb[64:128, k0 : k0 + 128],
                                 qt_sb[64:128, q0 : q0 + 512],
                                 start=True,
                                 stop=True,
                             )
                             emt = expm.tile([128, 2, 512], F32R, tag="expm")
                             nc.scalar.activation(
                                 out=emt.rearrange("p a t -> p (a t)"),
                                 in_=sct.rearrange("p a t -> p (a t)"),
                                 func=AF.Exp,
                                 bias=mb_sb[:, b, kt : kt + 1],
                                 scale=0.125,
                             )
                             st, sp = (kt == 0), (kt == NKT - 1)
                             gc = b * NKT + kt
                             nc.tensor.matmul(
                                 cxs[0], v_sb[0:64, gc, 0:65], emt[0:64, 0, :],
                                 start=st, stop=sp,
                             )
                             nc.tensor.matmul(
                                 cxs[1], v_sb[64:128, gc, 0:65], emt[64:128, 0, :],
                                 start=st, stop=sp,
                             )
                             nc.tensor.matmul(
                                 cxs[2], v_sb[0:64, gc, 65:130], emt[0:64, 1, :],
                                 start=st, stop=sp,
                             )
                             nc.tensor.matmul(
                                 cxs[3], v_sb[64:128, gc, 65:130], emt[64:128, 1, :],
                                 start=st, stop=sp,
                             )
                         tt = b * NQT + qt
                         for h in range(HC):
                             cxa, cxb = cxs[2 * h], cxs[2 * h + 1]
                             s1 = dv.tile([65, 512], F32, tag="s1")
                             nc.vector.tensor_copy(s1, cxa)
                             s2 = dv.tile([65, 512], F32, tag="s2")
                             nc.vector.tensor_add(s2, s1, cxb)
                             rr = dv.tile([1, 512], F32R, tag="rr")
                             nc.vector.reciprocal(rr, s2[0:1, :])
                             s2r = dv.tile([65, 512], F32R, tag="s2r")
                             nc.vector.tensor_copy(s2r, s2)
                             bct = attps.tile([65, 512], F32, tag="sc")
                             nc.tensor.matmul(bct, onesc, rr, start=True, stop=True)
                             cs = dv.tile([65, 512], F32R, tag="cs")
                             nc.vector.tensor_mul(cs, s2r, bct)
                             nc.sync.dma_start(
                                 out=ctx_sb[h * 64 : (h + 1) * 64, tt, :],
                                 in_=cs[1:65, :],
                             )

             # ---- phase O: output projection (partial; host sums cores) ----
             if "O" in phases:
              with (
                 tc.tile_pool(name="ops", bufs=3, space="PSUM") as ops,
                 tc.tile_pool(name="outsb", bufs=3) as outsb,
             ):
                 for tt in range(NTT):
                     for oc in range(NOC):
                         opt = ops.tile([128, 512], F32, tag="op")
                         nc.tensor.matmul(
                             opt, wo_sb[:, oc, :], ctx_sb[:, tt, :],
                             start=True, stop=True,
                         )
                         ob = outsb.tile([128, 512], F32, tag="ob")
                         nc.vector.tensor_scalar_add(ob, opt, bo_sb[:, oc : oc + 1])
                         nc.sync.dma_start(
                             out=outT[oc * 128 : (oc + 1) * 128, tt * 512 : (tt + 1) * 512],
                             in_=ob,
                         )

    nc.compile()
    _NC_CACHE[key] = nc
    return nc


def make_in_maps(query, key_value, mask, Wq, bq, Wk, bk, Wv, bv, Wo, bo):
    xqT = np.ascontiguousarray(query.reshape(T, E).T).astype(np.float32)
    xkT = np.ascontiguousarray(key_value.reshape(T, E).T).astype(np.float32)
    mb = np.where(mask != 0, 0.0, -1.0e5).astype(np.float32).reshape(B, NKT, 128)
    in_maps = []
    for c in range(N_CORES):
        sl = slice(c * JC, (c + 1) * JC)
        in_maps.append(
            {
                "xqT": xqT,
                "xkT": xkT,
                "wqT": np.ascontiguousarray(Wq[sl, :].T).astype(np.float32),
                "wkT": np.ascontiguousarray(Wk[sl, :].T).astype(np.float32),
                "wvT": np.ascontiguousarray(Wv[sl, :].T).astype(np.float32),
                "woT": np.ascontiguousarray(Wo[:, sl].T).astype(np.float32),
                "bq": bq[sl].reshape(JC, 1).astype(np.float32),
                "bk": bk[sl].reshape(JC, 1).astype(np.float32),
                "bv": bv[sl].reshape(JC, 1).astype(np.float32),
                # only core 0 adds bo so the host-side partial sum sees it once
                "bo": (
                    bo.reshape(NOC, 128).astype(np.float32)
                    if c == 0
                    else np.zeros((NOC, 128), np.float32)
                ),
                "mb": mb,
            }
        )
    return in_maps


def kernel(query, key_value, mask, Wq, bq, Wk, bk, Wv, bv, Wo, bo):
    nc = build()
    in_maps = make_in_maps(
        np.asarray(query), np.asarray(key_value), np.asarray(mask),
        np.asarray(Wq), np.asarray(bq), np.asarray(Wk), np.asarray(bk),
        np.asarray(Wv), np.asarray(bv), np.asarray(Wo), np.asarray(bo),
    )
    res = run_bass_kernel_spmd(nc, in_maps, list(range(N_CORES)))
    acc = np.zeros((E, T), np.float32)
    for c in range(N_CORES):
        acc += res.results[c]["outT"]
    return np.ascontiguousarray(acc.T).reshape(B, LQ, E).astype(np.float32)

